# revision 27
# baseline (speedup 1.0000x reference)
"""DeepSeek-V2 MoE grouped-GEMM expert FFN (SwiGLU) on 8 Trainium2 NeuronCores.

Expert-parallel: tokens are pre-sorted by expert; each core gets a set of
(expert weights, <=512-token tile) work items. All three GEMMs keep the
weights as the stationary (lhsT) operand and stream activations token-major:

  gate^T[n,tok] = sum_k  gate_w[k,n]^T @ x^T[k,tok]     (k over HIDDEN/128)
  act  = silu(gate^T) * up^T        (bf16)
  y^T[h,tok]   = sum_f  down_w[f,h]^T @ act[f,tok]      (f over INTER/128)

Weights are host-rearranged per (tile, out-block) into [128, nk*128] slabs so
every weight DMA is a single large linear transfer and the device consumes
weights in exactly streaming order (each weight element is used once).

Queue layout (both HWDGE queues; gpsimd SWDGE is avoided — it splits
>128-descriptor transfers into serialized sub-batches at ~60GB/s):
  sync  (HWDGE): gate/up weight slabs + down-proj slabs, in consume order
  scalar(HWDGE): x activations (prefetched a tile ahead) + output stores

Head: the tensor engine is warmed with dummy matmuls right after the
framework preamble (the PE p-state ramp overlaps the initial DMA fill), and
tile 0 processes n=0,1 with k-split accumulation (k0-7 sweep then k8-15
sweep over both n-blocks) so the PE saturates as soon as the first ~0.75MB
lands instead of waiting for the full 3MB gate/up working set.
Compute dtype bf16, accumulation fp32 in PSUM, output fp32.
"""

import sys

if "/opt/trn_rl_repo" not in sys.path:
    sys.path.insert(0, "/opt/trn_rl_repo")

import numpy as np
import ml_dtypes

N_CORES = 8
HIDDEN = 2048
INTER = 1408
TOK_TILE = 512
KT = HIDDEN // 128  # 16
FT = INTER // 128   # 11

_NC_CACHE = {}


def _build_nc(T):
    """Bass program for one core: T independent (weights, 512-token) work items."""
    import concourse.bacc as bacc
    import concourse.mybir as mybir
    import concourse.tile as tile

    bf16 = mybir.dt.bfloat16
    f32 = mybir.dt.float32

    PG = 2              # down-proj h-tiles per slab
    NP = KT // PG       # 8 slabs
    QL = 4              # x k-tiles per quad chunk (tiles t>0)
    KH = KT // 2        # k-split half

    nc = bacc.Bacc("TRN2", target_bir_lowering=False, debug=False)
    xt = nc.dram_tensor("xt", [T, KT, 128, TOK_TILE], bf16, kind="ExternalInput")
    guw = nc.dram_tensor("guw", [T, FT, 128, 2 * HIDDEN], bf16, kind="ExternalInput")
    dw = nc.dram_tensor("dw", [T, KT, 128, INTER], bf16, kind="ExternalInput")
    yt = nc.dram_tensor("yt", [T, KT, 128, TOK_TILE], bf16, kind="ExternalOutput")

    with tile.TileContext(nc) as tc:
        with (
            tc.tile_pool(name="xpool", bufs=13) as xpool,
            tc.tile_pool(name="wpool", bufs=5) as wpool,
            tc.tile_pool(name="apool", bufs=2 * FT) as apool,
            tc.tile_pool(name="spool", bufs=3) as spool,
            tc.tile_pool(name="opool", bufs=4) as opool,
            tc.tile_pool(name="dmy", bufs=1) as dmy,
            tc.tile_pool(name="psA", bufs=2, space="PSUM") as psA,
            tc.tile_pool(name="psB", bufs=3, space="PSUM") as psB,
        ):
            # PE p-state warmup: the tensor engine ramps 0.65->1.2->2.4 GHz
            # over ~3us of continuous work. Burn that ramp on dummy matmuls
            # while the first DMAs are still in flight. The memset rides the
            # vector engine (earliest-starting sequencer) so the first dummy
            # matmul issues ~1us sooner than a gpsimd memset allows.
            dmw = dmy.tile([128, 128], bf16, name="dmw", tag="dmw")
            dmx = dmy.tile([128, 256], bf16, name="dmx", tag="dmx")
            nc.gpsimd.memset(dmw[:], 0.5)
            nc.gpsimd.memset(dmx[:], 0.5)
            warm = psA.tile([128, 256], f32, name="warm", tag="warm", bufs=1)
            for _ in range(24):
                nc.tensor.matmul(warm[:], dmw[:], dmx[:], start=True, stop=True,
                                 skip_group_check=True)

            xk_ap = {}

            def xk(t, k):
                xc, j = xk_ap[(t, k)]
                return xc[:, j, :]

            def dummy_mm(cnt):
                for _ in range(cnt):
                    nc.tensor.matmul(warm[:], dmw[:], dmx[:], start=True,
                                     stop=True, skip_group_check=True)

            def load_x(t, k0, kl, eng):
                xc = xpool.tile([128, kl, TOK_TILE], bf16,
                                name=f"x_{t}_{k0}", tag="x")
                eng.dma_start(
                    xc[:], xt[t, k0:k0 + kl, :, :].rearrange("k r c -> r k c"))
                for j in range(kl):
                    xk_ap[(t, k0 + j)] = (xc, j)

            # tile 0 head fill: the DMA queues ramp ~40->267GB/s over the
            # first ~10us, and the sync queue ramps first, so the k=0,1 x
            # singles ride sync interleaved with the first gate chunks; the
            # remaining pairs ride scalar (idle until the first stores).

            H2 = KH * 128

            def pair_block(t, na, g_tiles, acts, bridge=None):
                """K-split processing of n-blocks (na, na+1): sweep k0-7 over
                both, then k8-15, leaving the PSUM groups open in between so
                the PE saturates on half-slab weight chunks."""
                pss = []
                for n in (na, na + 1):
                    psg = psA.tile([128, TOK_TILE], f32, name=f"psg_{t}_{n}", tag="psg")
                    psu = psA.tile([128, TOK_TILE], f32, name=f"psu_{t}_{n}", tag="psu")
                    pss.append((psg, psu))
                for half in range(2):
                    ks = range(half * KH, (half + 1) * KH)
                    for i, n in enumerate((na, na + 1)):
                        gt = g_tiles[i]
                        psg, psu = pss[i]
                        if bridge and i == 1 and half == 0 and 200 in bridge:
                            dummy_mm(bridge[200])
                        for k in ks:
                            nc.tensor.matmul(
                                psg[:], gt[:, k * 128:(k + 1) * 128], xk(t, k),
                                start=(k == 0), stop=(k == KT - 1),
                            )
                            if bridge and i == 0 and half == 0 and k in bridge:
                                dummy_mm(bridge[k])
                        for k in ks:
                            nc.tensor.matmul(
                                psu[:], gt[:, HIDDEN + k * 128:HIDDEN + (k + 1) * 128],
                                xk(t, k), start=(k == 0), stop=(k == KT - 1),
                            )
                            if bridge and i == 0 and half == 0 and (100 + k) in bridge:
                                dummy_mm(bridge[100 + k])
                        if half == 1:
                            sg = spool.tile([128, TOK_TILE], f32,
                                            name=f"sg_{t}_{n}", tag="sg")
                            nc.scalar.activation(
                                sg[:], psg[:], mybir.ActivationFunctionType.Silu)
                            at = apool.tile([128, TOK_TILE], bf16,
                                            name=f"act_{t}_{n}", tag="act")
                            nc.vector.tensor_mul(at[:], sg[:], psu[:])
                            acts.append(at)

            for t in range(T):
                dwts = []
                acts = []

                if t == 0:
                    # head: n=0..5 processed as k-split pairs. First gate
                    # chunks + x k0,k1 interleaved on sync (the fast-ramping
                    # queue); x pairs k2..15 on scalar; K1 halves of all six
                    # slabs on sync; K2 halves of n=2..5 on scalar (behind the
                    # x pairs) so the ramp-limited sync queue only has to
                    # deliver ~3.25MB by the n2/n3 deadline at ~33us.
                    g_head = []
                    for n in range(6):
                        gt = wpool.tile([128, 2 * HIDDEN], bf16,
                                        name=f"guw_0_{n}", tag="guw")
                        g_head.append(gt)
                    nc.sync.dma_start(g_head[0][:, 0:256], guw[0, 0, :, 0:256])
                    load_x(0, 0, 1, nc.sync)
                    nc.sync.dma_start(g_head[0][:, 256:H2], guw[0, 0, :, 256:H2])
                    load_x(0, 1, 1, nc.sync)
                    for c in range(1, KT // 2):
                        load_x(0, 2 * c, 2, nc.scalar)
                    nc.sync.dma_start(g_head[0][:, HIDDEN:HIDDEN + H2],
                                      guw[0, 0, :, HIDDEN:HIDDEN + H2])
                    nc.sync.dma_start(g_head[1][:, 0:H2], guw[0, 1, :, 0:H2])
                    nc.sync.dma_start(g_head[1][:, HIDDEN:HIDDEN + H2],
                                      guw[0, 1, :, HIDDEN:HIDDEN + H2])
                    for n in range(2):
                        nc.sync.dma_start(g_head[n][:, H2:HIDDEN],
                                          guw[0, n, :, H2:HIDDEN])
                        nc.sync.dma_start(g_head[n][:, HIDDEN + H2:],
                                          guw[0, n, :, HIDDEN + H2:])
                    for n in range(2, 5):
                        nc.sync.dma_start(g_head[n][:, 0:H2], guw[0, n, :, 0:H2])
                        nc.sync.dma_start(g_head[n][:, HIDDEN:HIDDEN + H2],
                                          guw[0, n, :, HIDDEN:HIDDEN + H2])
                    # n5 rides the otherwise-idle gpsimd SWDGE as a third DMA
                    # lane (each half is exactly 128 descriptors so SWDGE's
                    # >128-descriptor serialization never triggers). The lane
                    # is GATED on n0's K2 chunk having landed (~20us): before
                    # that the 16 shared SDMA engines are saturated and SWDGE
                    # traffic would starve the critical sync stream; after it
                    # they have headroom and n5's ~46-53us deadlines are easy.
                    gsink = dmy.tile([128, 16], bf16, name="gsink", tag="gsink")
                    nc.gpsimd.tensor_copy(gsink[:], g_head[0][:, H2:H2 + 16])
                    nc.gpsimd.dma_start(g_head[5][:, 0:H2], guw[0, 5, :, 0:H2])
                    nc.gpsimd.dma_start(g_head[5][:, HIDDEN:HIDDEN + H2],
                                        guw[0, 5, :, HIDDEN:HIDDEN + H2])
                    nc.gpsimd.dma_start(g_head[5][:, H2:HIDDEN],
                                        guw[0, 5, :, H2:HIDDEN])
                    nc.gpsimd.dma_start(g_head[5][:, HIDDEN + H2:],
                                        guw[0, 5, :, HIDDEN + H2:])
                    for n in range(2, 4):
                        nc.scalar.dma_start(g_head[n][:, H2:HIDDEN],
                                            guw[0, n, :, H2:HIDDEN])
                        nc.scalar.dma_start(g_head[n][:, HIDDEN + H2:],
                                            guw[0, n, :, HIDDEN + H2:])
                    nc.scalar.dma_start(g_head[4][:, H2:HIDDEN],
                                        guw[0, 4, :, H2:HIDDEN])
                    nc.scalar.dma_start(g_head[4][:, HIDDEN + H2:],
                                        guw[0, 4, :, HIDDEN + H2:])
                    pair_block(0, 0, g_head[0:2], acts,
                               bridge={1: 8, 3: 6, 5: 4, 7: 2, 200: 4})
                    pair_block(0, 2, g_head[2:4], acts)
                    pair_block(0, 4, g_head[4:6], acts)
                    n_start = 6
                else:
                    n_start = 0

                def load_dw(p, eng=nc.sync):
                    dwt = wpool.tile([128, PG, INTER], bf16,
                                     name=f"dw_{t}_{p}", tag="dw", bufs=8)
                    eng.dma_start(
                        dwt[:], dw[t, p * PG:(p + 1) * PG, :, :].rearrange("h r c -> r h c"))
                    dwts.append(dwt)

                if t == 0:
                    # first three down-proj slabs ride the scalar queue (idle
                    # in the 45-80us window) so they neither block tile 0's
                    # gate/up slabs on sync nor arrive late for the down phase
                    for p in range(3):
                        load_dw(p, nc.scalar)

                for n in range(n_start, FT):
                    guwt = wpool.tile([128, 2 * HIDDEN], bf16,
                                      name=f"guw_{t}_{n}", tag="guw")
                    nc.sync.dma_start(guwt[:], guw[t, n, :, :])

                    # down-proj slabs ride sync behind the gate/up stream
                    if n >= 3:
                        load_dw(n - 3)

                    # next tile's x quads ride scalar, early enough that the
                    # down-phase stores never queue behind their transfers
                    if t + 1 < T:
                        if t == 0 and n in (6, 7):
                            for qq in range(2):
                                load_x(t + 1, ((n - 6) * 2 + qq) * QL, QL, nc.scalar)
                        elif t > 0 and n in (2, 4, 6, 8):
                            load_x(t + 1, ((n - 2) // 2) * QL, QL, nc.scalar)

                    psg = psA.tile([128, TOK_TILE], f32, name=f"psg_{t}_{n}", tag="psg")
                    psu = psA.tile([128, TOK_TILE], f32, name=f"psu_{t}_{n}", tag="psu")
                    for k in range(KT):
                        nc.tensor.matmul(
                            psg[:], guwt[:, k * 128:(k + 1) * 128], xk(t, k),
                            start=(k == 0), stop=(k == KT - 1),
                        )
                    for k in range(KT):
                        nc.tensor.matmul(
                            psu[:], guwt[:, HIDDEN + k * 128:HIDDEN + (k + 1) * 128],
                            xk(t, k), start=(k == 0), stop=(k == KT - 1),
                        )

                    sg = spool.tile([128, TOK_TILE], f32, name=f"sg_{t}_{n}", tag="sg")
                    nc.scalar.activation(
                        sg[:], psg[:], mybir.ActivationFunctionType.Silu
                    )
                    at = apool.tile([128, TOK_TILE], bf16, name=f"act_{t}_{n}", tag="act")
                    nc.vector.tensor_mul(at[:], sg[:], psu[:])
                    acts.append(at)

                while len(dwts) < NP:
                    load_dw(len(dwts))

                for p in range(NP):
                    dwt = dwts[p]
                    for j in range(PG):
                        h = p * PG + j
                        if t == T - 1 and h == KT - 1:
                            # final h-block: two half-token chains so the last
                            # copy/store overlaps the second chain's matmuls;
                            # the two stores ride different HWDGE queues so
                            # their descriptor generations overlap
                            half = TOK_TILE // 2
                            for ci, eng in ((0, nc.scalar), (1, nc.sync)):
                                sl = slice(ci * half, (ci + 1) * half)
                                psy = psB.tile([128, half], f32,
                                               name=f"psy_{t}_{h}_{ci}", tag="psy")
                                for f in range(FT):
                                    nc.tensor.matmul(
                                        psy[:], dwt[:, j, f * 128:(f + 1) * 128],
                                        acts[f][:, sl],
                                        start=(f == 0), stop=(f == FT - 1),
                                    )
                                ot = opool.tile([128, half], bf16,
                                                name=f"o_{t}_{h}_{ci}", tag="o")
                                nc.vector.tensor_copy(ot[:], psy[:])
                                eng.dma_start(yt[t, h, :, sl], ot[:])
                        else:
                            psy = psB.tile([128, TOK_TILE], f32,
                                           name=f"psy_{t}_{h}", tag="psy")
                            for f in range(FT):
                                nc.tensor.matmul(
                                    psy[:], dwt[:, j, f * 128:(f + 1) * 128], acts[f][:],
                                    start=(f == 0), stop=(f == FT - 1),
                                )
                            ot = opool.tile([128, TOK_TILE], bf16, name=f"o_{t}_{h}", tag="o")
                            nc.vector.tensor_copy(ot[:], psy[:])
                            nc.scalar.dma_start(yt[t, h, :, :], ot[:])

    nc.compile()
    return nc


def _get_nc(T):
    if T not in _NC_CACHE:
        _NC_CACHE[T] = _build_nc(T)
    return _NC_CACHE[T]


def kernel(hidden_states, gate_w, up_w, down_w, group_sizes):
    from concourse.bass_utils import run_bass_kernel_spmd

    bf16 = ml_dtypes.bfloat16
    X = np.ascontiguousarray(np.asarray(hidden_states))
    gs = np.asarray(group_sizes).astype(np.int64)
    num_tokens, H = X.shape
    E, _, F = gate_w.shape
    assert H == HIDDEN and F == INTER

    # work-item list: (expert, row_start, nrows), rows grouped by expert
    tiles = []
    off = 0
    for e in range(E):
        m = int(gs[e])
        s = 0
        while s < m:
            nr = min(TOK_TILE, m - s)
            tiles.append((e, off + s, nr))
            s += nr
        off += m

    out = np.zeros((num_tokens, H), dtype=np.float32)
    if not tiles:
        return out
    while len(tiles) % N_CORES:
        tiles.append((tiles[0][0], 0, 0))  # dummy pad tile; output discarded
    T = len(tiles) // N_CORES

    Xb = X.astype(bf16)
    Gb = np.asarray(gate_w).astype(bf16)
    Ub = np.asarray(up_w).astype(bf16)
    Db = np.asarray(down_w).astype(bf16)

    # per-expert weight rearrangement (cached per expert within this call)
    gu_cache, d_cache = {}, {}

    def gu_r(e):
        if e not in gu_cache:
            g = Gb[e].reshape(KT, 128, FT, 128).transpose(2, 1, 0, 3).reshape(
                FT, 128, HIDDEN)
            u = Ub[e].reshape(KT, 128, FT, 128).transpose(2, 1, 0, 3).reshape(
                FT, 128, HIDDEN)
            gu_cache[e] = np.concatenate([g, u], axis=-1)
        return gu_cache[e]

    def d_r(e):
        if e not in d_cache:
            d_cache[e] = np.ascontiguousarray(
                Db[e].reshape(FT, 128, KT, 128).transpose(2, 1, 0, 3)
            ).reshape(KT, 128, INTER)
        return d_cache[e]

    in_maps = []
    for c in range(N_CORES):
        tl = tiles[c * T:(c + 1) * T]
        xtb = np.zeros((T, KT, 128, TOK_TILE), dtype=bf16)
        guwb = np.empty((T, FT, 128, 2 * HIDDEN), dtype=bf16)
        dwb = np.empty((T, KT, 128, INTER), dtype=bf16)
        for i, (e, r0, nr) in enumerate(tl):
            if nr:
                xtb[i, :, :, :nr] = Xb[r0:r0 + nr].T.reshape(KT, 128, nr)
            guwb[i] = gu_r(e)
            dwb[i] = d_r(e)
        in_maps.append({"xt": xtb, "guw": guwb, "dw": dwb})

    nc = _get_nc(T)
    res = run_bass_kernel_spmd(nc, in_maps, core_ids=list(range(N_CORES)))

    for c in range(N_CORES):
        ytc = res.results[c]["yt"]  # [T, KT, 128, TOK_TILE] f32
        for i, (e, r0, nr) in enumerate(tiles[c * T:(c + 1) * T]):
            if nr:
                out[r0:r0 + nr] = (
                    ytc[i].transpose(2, 0, 1).reshape(TOK_TILE, H)[:nr]
                )
    return out



# revision 28
# speedup vs baseline: 1.0132x; 1.0132x over previous
"""DeepSeek-V2 MoE grouped-GEMM expert FFN (SwiGLU) on 8 Trainium2 NeuronCores.

Expert-parallel: tokens are pre-sorted by expert; each core gets a set of
(expert weights, <=512-token tile) work items. All three GEMMs keep the
weights as the stationary (lhsT) operand and stream activations token-major:

  gate^T[n,tok] = sum_k  gate_w[k,n]^T @ x^T[k,tok]     (k over HIDDEN/128)
  act  = silu(gate^T) * up^T        (bf16)
  y^T[h,tok]   = sum_f  down_w[f,h]^T @ act[f,tok]      (f over INTER/128)

Weights are host-rearranged per (tile, out-block) into [128, nk*128] slabs so
every weight DMA is a single large linear transfer and the device consumes
weights in exactly streaming order (each weight element is used once).

Queue layout (both HWDGE queues; gpsimd SWDGE is avoided — it splits
>128-descriptor transfers into serialized sub-batches at ~60GB/s):
  sync  (HWDGE): gate/up weight slabs + down-proj slabs, in consume order
  scalar(HWDGE): x activations (prefetched a tile ahead) + output stores

Head: the tensor engine is warmed with dummy matmuls right after the
framework preamble (the PE p-state ramp overlaps the initial DMA fill), and
tile 0 processes n=0,1 with k-split accumulation (k0-7 sweep then k8-15
sweep over both n-blocks) so the PE saturates as soon as the first ~0.75MB
lands instead of waiting for the full 3MB gate/up working set.
Compute dtype bf16, accumulation fp32 in PSUM, output fp32.
"""

import sys

if "/opt/trn_rl_repo" not in sys.path:
    sys.path.insert(0, "/opt/trn_rl_repo")

import numpy as np
import ml_dtypes

N_CORES = 8
HIDDEN = 2048
INTER = 1408
TOK_TILE = 512
KT = HIDDEN // 128  # 16
FT = INTER // 128   # 11

_NC_CACHE = {}


def _build_nc(T):
    """Bass program for one core: T independent (weights, 512-token) work items."""
    import concourse.bacc as bacc
    import concourse.mybir as mybir
    import concourse.tile as tile

    bf16 = mybir.dt.bfloat16
    f32 = mybir.dt.float32

    PG = 2              # down-proj h-tiles per slab
    NP = KT // PG       # 8 slabs
    QL = 4              # x k-tiles per quad chunk (tiles t>0)
    KH = KT // 2        # k-split half

    nc = bacc.Bacc("TRN2", target_bir_lowering=False, debug=False)
    xt = nc.dram_tensor("xt", [T, KT, 128, TOK_TILE], bf16, kind="ExternalInput")
    guw = nc.dram_tensor("guw", [T, FT, 128, 2 * HIDDEN], bf16, kind="ExternalInput")
    dw = nc.dram_tensor("dw", [T, KT, 128, INTER], bf16, kind="ExternalInput")
    yt = nc.dram_tensor("yt", [T, KT, 128, TOK_TILE], bf16, kind="ExternalOutput")

    with tile.TileContext(nc) as tc:
        with (
            tc.tile_pool(name="xpool", bufs=13) as xpool,
            tc.tile_pool(name="wpool", bufs=5) as wpool,
            tc.tile_pool(name="apool", bufs=2 * FT) as apool,
            tc.tile_pool(name="spool", bufs=3) as spool,
            tc.tile_pool(name="opool", bufs=4) as opool,
            tc.tile_pool(name="dmy", bufs=1) as dmy,
            tc.tile_pool(name="psA", bufs=2, space="PSUM") as psA,
            tc.tile_pool(name="psB", bufs=3, space="PSUM") as psB,
        ):
            # PE p-state warmup: the tensor engine ramps 0.65->1.2->2.4 GHz
            # over ~3us of continuous work. Burn that ramp on dummy matmuls
            # while the first DMAs are still in flight. The memset rides the
            # vector engine (earliest-starting sequencer) so the first dummy
            # matmul issues ~1us sooner than a gpsimd memset allows.
            dmw = dmy.tile([128, 128], bf16, name="dmw", tag="dmw")
            dmx = dmy.tile([128, 256], bf16, name="dmx", tag="dmx")
            nc.gpsimd.memset(dmw[:], 0.5)
            nc.gpsimd.memset(dmx[:], 0.5)
            warm = psA.tile([128, 256], f32, name="warm", tag="warm", bufs=1)
            for _ in range(16):
                nc.tensor.matmul(warm[:], dmw[:], dmx[:], start=True, stop=True,
                                 skip_group_check=True)

            xk_ap = {}

            def xk(t, k):
                xc, j = xk_ap[(t, k)]
                return xc[:, j, :]

            def dummy_mm(cnt):
                for _ in range(cnt):
                    nc.tensor.matmul(warm[:], dmw[:], dmx[:], start=True,
                                     stop=True, skip_group_check=True)

            def load_x(t, k0, kl, eng):
                xc = xpool.tile([128, kl, TOK_TILE], bf16,
                                name=f"x_{t}_{k0}", tag="x")
                eng.dma_start(
                    xc[:], xt[t, k0:k0 + kl, :, :].rearrange("k r c -> r k c"))
                for j in range(kl):
                    xk_ap[(t, k0 + j)] = (xc, j)

            # tile 0 head fill: the DMA queues ramp ~40->267GB/s over the
            # first ~10us, and the sync queue ramps first, so the k=0,1 x
            # singles ride sync interleaved with the first gate chunks; the
            # remaining pairs ride scalar (idle until the first stores).

            H2 = KH * 128

            def pair_block(t, na, g_tiles, acts, bridge=None):
                """K-split processing of n-blocks (na, na+1): sweep k0-7 over
                both, then k8-15, leaving the PSUM groups open in between so
                the PE saturates on half-slab weight chunks."""
                pss = []
                for n in (na, na + 1):
                    psg = psA.tile([128, TOK_TILE], f32, name=f"psg_{t}_{n}", tag="psg")
                    psu = psA.tile([128, TOK_TILE], f32, name=f"psu_{t}_{n}", tag="psu")
                    pss.append((psg, psu))
                for half in range(2):
                    ks = range(half * KH, (half + 1) * KH)
                    for i, n in enumerate((na, na + 1)):
                        gt = g_tiles[i]
                        psg, psu = pss[i]
                        if bridge and i == 1 and half == 0 and 200 in bridge:
                            dummy_mm(bridge[200])
                        for k in ks:
                            nc.tensor.matmul(
                                psg[:], gt[:, k * 128:(k + 1) * 128], xk(t, k),
                                start=(k == 0), stop=(k == KT - 1),
                            )
                            if bridge and i == 0 and half == 0 and k in bridge:
                                dummy_mm(bridge[k])
                        for k in ks:
                            nc.tensor.matmul(
                                psu[:], gt[:, HIDDEN + k * 128:HIDDEN + (k + 1) * 128],
                                xk(t, k), start=(k == 0), stop=(k == KT - 1),
                            )
                            if bridge and i == 0 and half == 0 and (100 + k) in bridge:
                                dummy_mm(bridge[100 + k])
                        if half == 1:
                            sg = spool.tile([128, TOK_TILE], f32,
                                            name=f"sg_{t}_{n}", tag="sg")
                            nc.scalar.activation(
                                sg[:], psg[:], mybir.ActivationFunctionType.Silu)
                            at = apool.tile([128, TOK_TILE], bf16,
                                            name=f"act_{t}_{n}", tag="act")
                            nc.vector.tensor_mul(at[:], sg[:], psu[:])
                            acts.append(at)

            for t in range(T):
                dwts = []
                acts = []

                if t == 0:
                    # head: n=0..5 processed as k-split pairs. First gate
                    # chunks + x k0,k1 interleaved on sync (the fast-ramping
                    # queue); x pairs k2..15 on scalar; K1 halves of all six
                    # slabs on sync; K2 halves of n=2..5 on scalar (behind the
                    # x pairs) so the ramp-limited sync queue only has to
                    # deliver ~3.25MB by the n2/n3 deadline at ~33us.
                    g_head = []
                    for n in range(6):
                        gt = wpool.tile([128, 2 * HIDDEN], bf16,
                                        name=f"guw_0_{n}", tag="guw")
                        g_head.append(gt)
                    nc.sync.dma_start(g_head[0][:, 0:256], guw[0, 0, :, 0:256])
                    load_x(0, 0, 1, nc.sync)
                    nc.sync.dma_start(g_head[0][:, 256:H2], guw[0, 0, :, 256:H2])
                    load_x(0, 1, 1, nc.sync)
                    for c in range(1, KT // 2):
                        load_x(0, 2 * c, 2, nc.scalar)
                    nc.sync.dma_start(g_head[0][:, HIDDEN:HIDDEN + H2],
                                      guw[0, 0, :, HIDDEN:HIDDEN + H2])
                    nc.sync.dma_start(g_head[1][:, 0:H2], guw[0, 1, :, 0:H2])
                    nc.sync.dma_start(g_head[1][:, HIDDEN:HIDDEN + H2],
                                      guw[0, 1, :, HIDDEN:HIDDEN + H2])
                    for n in range(2):
                        nc.sync.dma_start(g_head[n][:, H2:HIDDEN],
                                          guw[0, n, :, H2:HIDDEN])
                        nc.sync.dma_start(g_head[n][:, HIDDEN + H2:],
                                          guw[0, n, :, HIDDEN + H2:])
                    for n in range(2, 6):
                        nc.sync.dma_start(g_head[n][:, 0:H2], guw[0, n, :, 0:H2])
                        nc.sync.dma_start(g_head[n][:, HIDDEN:HIDDEN + H2],
                                          guw[0, n, :, HIDDEN:HIDDEN + H2])
                    for n in range(2, 4):
                        nc.sync.dma_start(g_head[n][:, H2:HIDDEN],
                                          guw[0, n, :, H2:HIDDEN])
                        nc.sync.dma_start(g_head[n][:, HIDDEN + H2:],
                                          guw[0, n, :, HIDDEN + H2:])
                    for n in range(4, 6):
                        nc.scalar.dma_start(g_head[n][:, H2:HIDDEN],
                                            guw[0, n, :, H2:HIDDEN])
                        nc.scalar.dma_start(g_head[n][:, HIDDEN + H2:],
                                            guw[0, n, :, HIDDEN + H2:])
                    pair_block(0, 0, g_head[0:2], acts,
                               bridge={1: 8, 3: 6, 5: 4, 7: 2, 200: 4})
                    pair_block(0, 2, g_head[2:4], acts)
                    pair_block(0, 4, g_head[4:6], acts)
                    n_start = 6
                else:
                    n_start = 0

                def load_dw(p, eng=nc.sync):
                    dwt = wpool.tile([128, PG, INTER], bf16,
                                     name=f"dw_{t}_{p}", tag="dw", bufs=8)
                    eng.dma_start(
                        dwt[:], dw[t, p * PG:(p + 1) * PG, :, :].rearrange("h r c -> r h c"))
                    dwts.append(dwt)

                if t == 0:
                    # first three down-proj slabs ride the scalar queue (idle
                    # in the 45-80us window) so they neither block tile 0's
                    # gate/up slabs on sync nor arrive late for the down phase
                    for p in range(3):
                        load_dw(p, nc.scalar)

                for n in range(n_start, FT):
                    guwt = wpool.tile([128, 2 * HIDDEN], bf16,
                                      name=f"guw_{t}_{n}", tag="guw")
                    nc.sync.dma_start(guwt[:], guw[t, n, :, :])

                    # down-proj slabs ride sync behind the gate/up stream
                    if n >= 3:
                        load_dw(n - 3)

                    # next tile's x quads ride scalar, early enough that the
                    # down-phase stores never queue behind their transfers
                    if t + 1 < T:
                        if t == 0 and n in (6, 7):
                            for qq in range(2):
                                load_x(t + 1, ((n - 6) * 2 + qq) * QL, QL, nc.scalar)
                        elif t > 0 and n in (2, 4, 6, 8):
                            load_x(t + 1, ((n - 2) // 2) * QL, QL, nc.scalar)

                    psg = psA.tile([128, TOK_TILE], f32, name=f"psg_{t}_{n}", tag="psg")
                    psu = psA.tile([128, TOK_TILE], f32, name=f"psu_{t}_{n}", tag="psu")
                    for k in range(KT):
                        nc.tensor.matmul(
                            psg[:], guwt[:, k * 128:(k + 1) * 128], xk(t, k),
                            start=(k == 0), stop=(k == KT - 1),
                        )
                    for k in range(KT):
                        nc.tensor.matmul(
                            psu[:], guwt[:, HIDDEN + k * 128:HIDDEN + (k + 1) * 128],
                            xk(t, k), start=(k == 0), stop=(k == KT - 1),
                        )

                    sg = spool.tile([128, TOK_TILE], f32, name=f"sg_{t}_{n}", tag="sg")
                    nc.scalar.activation(
                        sg[:], psg[:], mybir.ActivationFunctionType.Silu
                    )
                    at = apool.tile([128, TOK_TILE], bf16, name=f"act_{t}_{n}", tag="act")
                    nc.vector.tensor_mul(at[:], sg[:], psu[:])
                    acts.append(at)

                while len(dwts) < NP:
                    load_dw(len(dwts))

                for p in range(NP):
                    dwt = dwts[p]
                    for j in range(PG):
                        h = p * PG + j
                        if t == T - 1 and h == KT - 1:
                            # final h-block: two half-token chains so the last
                            # copy/store overlaps the second chain's matmuls;
                            # the two stores ride different HWDGE queues so
                            # their descriptor generations overlap
                            half = TOK_TILE // 2
                            for ci, eng in ((0, nc.scalar), (1, nc.sync)):
                                sl = slice(ci * half, (ci + 1) * half)
                                psy = psB.tile([128, half], f32,
                                               name=f"psy_{t}_{h}_{ci}", tag="psy")
                                for f in range(FT):
                                    nc.tensor.matmul(
                                        psy[:], dwt[:, j, f * 128:(f + 1) * 128],
                                        acts[f][:, sl],
                                        start=(f == 0), stop=(f == FT - 1),
                                    )
                                ot = opool.tile([128, half], bf16,
                                                name=f"o_{t}_{h}_{ci}", tag="o")
                                nc.vector.tensor_copy(ot[:], psy[:])
                                eng.dma_start(yt[t, h, :, sl], ot[:])
                        else:
                            psy = psB.tile([128, TOK_TILE], f32,
                                           name=f"psy_{t}_{h}", tag="psy")
                            for f in range(FT):
                                nc.tensor.matmul(
                                    psy[:], dwt[:, j, f * 128:(f + 1) * 128], acts[f][:],
                                    start=(f == 0), stop=(f == FT - 1),
                                )
                            ot = opool.tile([128, TOK_TILE], bf16, name=f"o_{t}_{h}", tag="o")
                            nc.vector.tensor_copy(ot[:], psy[:])
                            nc.scalar.dma_start(yt[t, h, :, :], ot[:])

    nc.compile()
    return nc


def _get_nc(T):
    if T not in _NC_CACHE:
        _NC_CACHE[T] = _build_nc(T)
    return _NC_CACHE[T]


def kernel(hidden_states, gate_w, up_w, down_w, group_sizes):
    from concourse.bass_utils import run_bass_kernel_spmd

    bf16 = ml_dtypes.bfloat16
    X = np.ascontiguousarray(np.asarray(hidden_states))
    gs = np.asarray(group_sizes).astype(np.int64)
    num_tokens, H = X.shape
    E, _, F = gate_w.shape
    assert H == HIDDEN and F == INTER

    # work-item list: (expert, row_start, nrows), rows grouped by expert
    tiles = []
    off = 0
    for e in range(E):
        m = int(gs[e])
        s = 0
        while s < m:
            nr = min(TOK_TILE, m - s)
            tiles.append((e, off + s, nr))
            s += nr
        off += m

    out = np.zeros((num_tokens, H), dtype=np.float32)
    if not tiles:
        return out
    while len(tiles) % N_CORES:
        tiles.append((tiles[0][0], 0, 0))  # dummy pad tile; output discarded
    T = len(tiles) // N_CORES

    Xb = X.astype(bf16)
    Gb = np.asarray(gate_w).astype(bf16)
    Ub = np.asarray(up_w).astype(bf16)
    Db = np.asarray(down_w).astype(bf16)

    # per-expert weight rearrangement (cached per expert within this call)
    gu_cache, d_cache = {}, {}

    def gu_r(e):
        if e not in gu_cache:
            g = Gb[e].reshape(KT, 128, FT, 128).transpose(2, 1, 0, 3).reshape(
                FT, 128, HIDDEN)
            u = Ub[e].reshape(KT, 128, FT, 128).transpose(2, 1, 0, 3).reshape(
                FT, 128, HIDDEN)
            gu_cache[e] = np.concatenate([g, u], axis=-1)
        return gu_cache[e]

    def d_r(e):
        if e not in d_cache:
            d_cache[e] = np.ascontiguousarray(
                Db[e].reshape(FT, 128, KT, 128).transpose(2, 1, 0, 3)
            ).reshape(KT, 128, INTER)
        return d_cache[e]

    in_maps = []
    for c in range(N_CORES):
        tl = tiles[c * T:(c + 1) * T]
        xtb = np.zeros((T, KT, 128, TOK_TILE), dtype=bf16)
        guwb = np.empty((T, FT, 128, 2 * HIDDEN), dtype=bf16)
        dwb = np.empty((T, KT, 128, INTER), dtype=bf16)
        for i, (e, r0, nr) in enumerate(tl):
            if nr:
                xtb[i, :, :, :nr] = Xb[r0:r0 + nr].T.reshape(KT, 128, nr)
            guwb[i] = gu_r(e)
            dwb[i] = d_r(e)
        in_maps.append({"xt": xtb, "guw": guwb, "dw": dwb})

    nc = _get_nc(T)
    res = run_bass_kernel_spmd(nc, in_maps, core_ids=list(range(N_CORES)))

    for c in range(N_CORES):
        ytc = res.results[c]["yt"]  # [T, KT, 128, TOK_TILE] f32
        for i, (e, r0, nr) in enumerate(tiles[c * T:(c + 1) * T]):
            if nr:
                out[r0:r0 + nr] = (
                    ytc[i].transpose(2, 0, 1).reshape(TOK_TILE, H)[:nr]
                )
    return out

